# revision 41
# baseline (speedup 1.0000x reference)
"""Biaffine kernel for Trainium2, data-parallel over batch across 8 NeuronCores.

Math (reference):
  Ha = [H, 1]                                   # [B, N, d+1]
  out[b,x,y] = D[b,x,:] @ U @ Ha[b,y,:]  +  Ha[b,x,:]@W[:d+1]  +  D[b,y,:]@W[d+1:]

Decomposition used here (per batch b):
  U1 = U[:, :d]  (d x d),  u2 = U[:, d]
  G_b  = U1 @ H_b^T                             # [d, N]   (step 1, PE)
  S_b  = D_b @ G_b                              # [N, N]   (step 2, PE)
  rowvec[x] = D_b[x,:]@u2 + H_b[x,:]@W[:d]      # via skinny matmuls (vec)
  colvec[y] = D_b[y,:]@W[d+1:] + W[d]
  out_b = S_b + rowvec x 1 + 1 x colvec         # folded into step 2 as a K=2 matmul

Schedule: per pair p of batches -- vec(p), step1(p), step2(2p), step2(2p+1).
Pair 0's step1 b01=0 runs jc-major across 8 PSUM banks so the PE consumes
(ujt chunk, ht chunk) pairs in DMA arrival order; warm-up matmuls on a
memset tile cover the DMA lead-in and keep the HAM clock gate hot.
DMA paths: critical pair-0 stream on the Sync HWDGE ring, bulk prefetch on
the GpSimd SWDGE ring, outputs on the Scalar HWDGE ring.
All matmuls in bf16 with fp32 PSUM accumulation.
"""

import sys

for _p in ("/opt/trn_rl_repo", "/root/.axon_site/_ro/trn_rl_repo"):
    if _p not in sys.path:
        sys.path.append(_p)

import ml_dtypes
import numpy as np

B, N, DD = 64, 512, 1024
NCORES = 8
BPC = B // NCORES  # batches per core
P = 128
KC = DD // P  # 8 contraction chunks of 128
XC = N // P  # 4 output-row chunks of 128
NP = BPC // 2  # batch pairs
NWARM = 80  # warm-up matmuls issued before any DMA-dependent work

BF16 = ml_dtypes.bfloat16

LAST_RESULT = None  # BassKernelResults of the most recent run (for test.py)


def _ensure_axon_ntff_hook():
    """Provide antenv.axon_hooks if the image lacks it, so trace=True works
    under axon. No-op when the real module exists or the .so is absent."""
    try:
        import antenv.axon_hooks  # noqa: F401
        return
    except ImportError:
        pass
    import contextlib
    import ctypes
    import os
    import types

    holder = {"hook": None, "built": False}

    def _build_hook():
        so_path = "/opt/axon/libaxon_pjrt.so"
        if not os.path.exists(so_path):
            return None
        lib = ctypes.CDLL(so_path)
        if not hasattr(lib, "axon_start_nrt_profile"):
            return None
        lib.axon_start_nrt_profile.argtypes = [
            ctypes.POINTER(ctypes.c_int64),
            ctypes.c_size_t,
        ]
        lib.axon_start_nrt_profile.restype = ctypes.c_int64
        lib.axon_stop_nrt_profile.argtypes = [ctypes.c_char_p]
        lib.axon_stop_nrt_profile.restype = ctypes.c_int64

        @contextlib.contextmanager
        def _hook(output_dir, device_ids):
            import jax

            jax.devices()
            if device_ids:
                ids = (ctypes.c_int64 * len(device_ids))(*device_ids)
                rc = lib.axon_start_nrt_profile(ids, len(device_ids))
            else:
                rc = lib.axon_start_nrt_profile(None, 0)
            if rc != 0:
                raise RuntimeError(f"axon_start_nrt_profile rc={rc}")
            try:
                yield
            finally:
                n = lib.axon_stop_nrt_profile(str(output_dir).encode())
                print(f"ntff profile: {n} file(s) -> {output_dir}")

        return _hook

    def set_axon_ntff_profile_hook(h):
        holder["hook"] = h
        holder["built"] = True

    def get_axon_ntff_profile_hook():
        if not holder["built"]:
            holder["hook"] = _build_hook()
            holder["built"] = True
        return holder["hook"]

    mod = types.ModuleType("antenv.axon_hooks")
    mod.set_axon_ntff_profile_hook = set_axon_ntff_profile_hook
    mod.get_axon_ntff_profile_hook = get_axon_ntff_profile_hook
    sys.modules["antenv.axon_hooks"] = mod
    try:
        import antenv

        antenv.axon_hooks = mod
    except ImportError:
        pass


def _build_bass(c_const: float):
    import concourse.mybir as mybir
    import concourse.tile as tile
    from concourse import bacc
    from contextlib import ExitStack

    bf = mybir.dt.bfloat16
    f32 = mybir.dt.float32
    ACT = mybir.ActivationFunctionType

    nc = bacc.Bacc("TRN2")
    dtr_h = nc.dram_tensor("dtr", [BPC, P, KC, N], bf, kind="ExternalInput")
    htr_h = nc.dram_tensor("htr", [NP, KC, P, 2 * N], bf, kind="ExternalInput")
    ujt_h = nc.dram_tensor("ujt", [P, KC, DD], bf, kind="ExternalInput")
    vpr_h = nc.dram_tensor("vpr", [P, KC, 2], bf, kind="ExternalInput")
    u2r_h = nc.dram_tensor("u2r", [P, KC], f32, kind="ExternalInput")
    # per-partition (scale, bias) pairs for the lvec/rvec assembly ACT ops:
    # cols = (scale_l, bias_l, scale_r, bias_r)
    cst_h = nc.dram_tensor("cst", [2, 4], f32, kind="ExternalInput")
    out_h = nc.dram_tensor("out", [BPC, N, N], f32, kind="ExternalOutput")

    with tile.TileContext(nc) as tc, ExitStack() as ctx:
        const_pool = ctx.enter_context(tc.tile_pool(name="const", bufs=1))
        vp_s = const_pool.tile([P, KC, 2], bf, name="vp_s")
        u2_s = const_pool.tile([P, KC], f32, name="u2_s")
        ujt_s = const_pool.tile([P, KC, DD], bf, name="ujt_s")
        warm_s = const_pool.tile([P, P], bf, name="warm_s")
        cst_s = const_pool.tile([2, 4], f32, name="cst_s")
        # persistent row/col vector tiles (4-deep: 2 batches/pair x 2 pairs
        # in flight); lvec = [rowH; 1], rvec = [1; colvec+c], both rows
        # written by one 2-partition ACT op per batch
        lvecs = [const_pool.tile([2, N], bf, name=f"lvec{i}") for i in range(4)]
        rvecs = [const_pool.tile([2, N], bf, name=f"rvec{i}") for i in range(4)]

        hpool = ctx.enter_context(tc.tile_pool(name="hpool", bufs=3))
        dpool = ctx.enter_context(tc.tile_pool(name="dpool", bufs=6))
        gpool = ctx.enter_context(tc.tile_pool(name="gpool", bufs=1))
        opool = ctx.enter_context(tc.tile_pool(name="opool", bufs=4))
        pspool = ctx.enter_context(tc.tile_pool(name="ps", bufs=8, space="PSUM"))

        # engine-local init (no DMA deps): warm tile
        nc.gpsimd.memset(warm_s[:], 0.125)
        gate_s = const_pool.tile([1, 1], bf, name="gate_s")

        # warm-up matmuls (N=128): cover the DMA lead-in at fine granularity
        # and flip the HAM clock gate
        wps = pspool.tile([P, N], f32, name="warm_ps", tag="ps")
        for i in range(NWARM):
            nc.tensor.matmul(
                wps[:, 0:P], lhsT=warm_s[:], rhs=warm_s[:],
                start=(i == 0), stop=(i == NWARM - 1),
            )

        # ---- critical-path pair-0 DMA in big consumption-ordered chunks
        # across both HWDGE rings: ujt/consts/dt on Sync, ht0 on Scalar
        ht_tiles = {}
        dt_tiles = {}
        ht0 = hpool.tile([P, KC, 2 * N], bf, name="ht0", tag="ht")
        ht_tiles[0] = ht0
        for jc0 in range(0, KC, 4):
            nc.scalar.dma_start(
                ht0[:, jc0:jc0 + 4, 0:N],
                htr_h[0, jc0:jc0 + 4, :, 0:N].rearrange("j p n -> p j n"),
            )
        for jc0 in range(0, KC, 4):
            nc.scalar.dma_start(
                ht0[:, jc0:jc0 + 4, N:2 * N],
                htr_h[0, jc0:jc0 + 4, :, N:2 * N].rearrange("j p n -> p j n"),
            )
        for jc0 in range(0, KC, 2):
            nc.sync.dma_start(ujt_s[:, jc0:jc0 + 2, :], ujt_h[:, jc0:jc0 + 2, :])
        nc.sync.dma_start(u2_s[:], u2r_h[:])
        nc.sync.dma_start(vp_s[:], vpr_h[:])
        nc.sync.dma_start(cst_s[:], cst_h[:])

        def load_dt(engine, b, gate=None):
            dt = dpool.tile([P, KC, N], bf, name=f"dt{b}", tag="dt")
            if gate is not None:
                nc.gpsimd.tensor_copy(dt[0:1, 0, 0:1], gate)
            engine.dma_start(dt[:], dtr_h[b])
            dt_tiles[b] = dt

        load_dt(nc.sync, 0)
        load_dt(nc.sync, 1)

        # ---- bulk prefetch (GpSimd SWDGE ring). Each transfer is gated
        # behind pair 0's critical ujt/u2 loads via a 1-element corner
        # write -- a WAW dependency the scheduler cannot hoist.
        gate1 = u2_s[0:1, 0:1]

        def load_ht(p, g):
            ht = hpool.tile([P, KC, 2 * N], bf, name=f"ht{p}", tag="ht")
            for jc0 in range(0, KC, 4):
                nc.gpsimd.tensor_copy(ht[0:1, jc0, 0:1], g)
                nc.gpsimd.dma_start(
                    ht[:, jc0:jc0 + 4, :],
                    htr_h[p, jc0:jc0 + 4].rearrange("j p n -> p j n"),
                )
            ht_tiles[p] = ht

        for p in range(1, NP):
            load_ht(p, gate1)
            load_dt(nc.gpsimd, 2 * p, gate=gate1)
            load_dt(nc.gpsimd, 2 * p + 1, gate=gate1)

        def vec_pair(p):
            # Four M=2 contraction streams (weights [wh, wd]) on distinct
            # 32-col PE groups, interleaved per k so they run concurrently.
            # ht streams yield rowH at partitions {0, 64} (+junk) in bank vA,
            # dt streams yield colD at partitions {33, 97} in bank vB -- two
            # banks so the lvec (DVE) and rvec (ACT) assemblies run parallel.
            ht = ht_tiles[p]
            d0, d1 = dt_tiles[2 * p], dt_tiles[2 * p + 1]
            # one PSUM bank per stream so each batch's assembly can start
            # as soon as its own streams stop
            vA0 = pspool.tile([2, N], f32, name=f"vA0_{p}", tag="ps")
            vB0 = pspool.tile([34, N], f32, name=f"vB0_{p}", tag="ps")
            vA1 = pspool.tile([66, N], f32, name=f"vA1_{p}", tag="ps")
            vB1 = pspool.tile([98, N], f32, name=f"vB1_{p}", tag="ps")
            vts = (vA0, vB0, vA1, vB1)
            POS = (0, 32, 64, 96)  # (rowH b0, colD b0, rowH b1, colD b1)

            def out_rhs(s, k):
                if s == 0:
                    return vA0, ht[:, k, 0:N]
                if s == 1:
                    return vB0, d0[:, k, :]
                if s == 2:
                    return vA1, ht[:, k, N:2 * N]
                return vB1, d1[:, k, :]

            for k in range(KC):
                for s in range(4):
                    vt, rhs = out_rhs(s, k)
                    nc.tensor.matmul(
                        vt[POS[s]:POS[s] + 2, :],
                        lhsT=vp_s[:, k, 0:2],
                        rhs=rhs,
                        start=(k == 0), stop=(k == KC - 1),
                        tile_position=(0, POS[s]),
                        skip_group_check=True,
                    )
            # assemble lvec = [rowH; 1] on DVE and rvec = [1; colD+c] on ACT
            for b01 in range(2):
                b = 2 * p + b01
                lv, rv = lvecs[b % 4], rvecs[b % 4]
                lp, rp = POS[2 * b01], POS[2 * b01 + 1]
                nc.vector.tensor_scalar(
                    lv[0:2, :], vts[2 * b01][lp:lp + 2, :],
                    cst_s[0:2, 0:1], cst_s[0:2, 1:2],
                    op0=mybir.AluOpType.mult, op1=mybir.AluOpType.add,
                )
                nc.scalar.activation(
                    rv[0:2, :], vts[2 * b01 + 1][rp:rp + 2, :], ACT.Identity,
                    scale=cst_s[0:2, 2:3], bias=cst_s[0:2, 3:4],
                )

        def s1_phase_a0(g2):
            # pair 0 b01=0 jc-major across 8 PSUM banks: consumes
            # (ujt[jc], ht[jc]) chunk pairs in DMA arrival order
            ht = ht_tiles[0]
            banks = [
                pspool.tile([P, N], f32, name=f"gA{ic}", tag="ps")
                for ic in range(KC)
            ]
            for jc in range(KC):
                for ic in range(KC):
                    nc.tensor.matmul(
                        banks[ic][:],
                        lhsT=ujt_s[:, jc, ic * P:(ic + 1) * P],
                        rhs=ht[:, jc, 0:N],
                        start=(jc == 0), stop=(jc == KC - 1),
                    )
            for ic in range(KC):
                nc.vector.tensor_scalar_add(
                    g2[:, ic, 0:N], banks[ic][:], u2_s[:, ic:ic + 1]
                )

        def step1_pair(p, g2, b01s):
            # G[i, y] = U1 @ H_b^T per batch; the PSUM->SBUF copy adds u2[i]
            # per partition, folding D.u2 into step 2.
            ht = ht_tiles[p]
            for ic in range(KC):
                for b01 in b01s:
                    g_ps = pspool.tile([P, N], f32, name=f"gps{p}_{ic}_{b01}", tag="ps")
                    for jc in range(KC):
                        nc.tensor.matmul(
                            g_ps[:],
                            lhsT=ujt_s[:, jc, ic * P:(ic + 1) * P],
                            rhs=ht[:, jc, b01 * N:(b01 + 1) * N],
                            start=(jc == 0), stop=(jc == KC - 1),
                        )
                    nc.vector.tensor_scalar_add(
                        g2[:, ic, b01 * N:(b01 + 1) * N], g_ps[:], u2_s[:, ic:ic + 1]
                    )

        def step2(b, g2, rank2_first=True):
            b01 = b % 2
            dt = dt_tiles[b]
            lv, rv = lvecs[b % 4], rvecs[b % 4]
            # rank-2 terms (rowvec[x]*1 + 1*colvec[y]) first, batched: the
            # K=2 weight loads then never interrupt the uniform K=128 stream.
            # (For batch 0 the lvec/rvec assembly has just been issued, so
            # the rank-2 MMs go last there to avoid stalling on it.)
            banks = []
            for xc in range(XC):
                s_ps = pspool.tile([P, N], f32, name=f"sps{b}_{xc}", tag="ps")
                banks.append(s_ps)
                if rank2_first:
                    nc.tensor.matmul(
                        s_ps[:],
                        lhsT=lv[:, xc * P:(xc + 1) * P],
                        rhs=rv[:, :],
                        start=True, stop=False,
                        skip_group_check=True,
                    )
            for xc in range(XC):
                s_ps = banks[xc]
                for ic in range(KC):
                    nc.tensor.matmul(
                        s_ps[:],
                        lhsT=dt[:, ic, xc * P:(xc + 1) * P],
                        rhs=g2[:, ic, b01 * N:(b01 + 1) * N],
                        start=(not rank2_first and ic == 0),
                        stop=(rank2_first and ic == KC - 1),
                        skip_group_check=True,
                    )
                if not rank2_first:
                    nc.tensor.matmul(
                        s_ps[:],
                        lhsT=lv[:, xc * P:(xc + 1) * P],
                        rhs=rv[:, :],
                        start=False, stop=True,
                        skip_group_check=True,
                    )
                o_t = opool.tile([P, N], f32, name=f"o{b}_{xc}", tag="o")
                if b == BPC - 1 and xc >= 2:
                    # final two tiles: alternate engines and halve the
                    # copy+DMA chains so the epilogue drains in parallel
                    cp, dma = (
                        (nc.vector.tensor_copy, nc.sync.dma_start)
                        if xc == 2 else
                        (lambda o, s: nc.scalar.activation(o, s, ACT.Copy),
                         nc.scalar.dma_start)
                    )
                    for h0 in (0, P // 2):
                        h1 = h0 + P // 2
                        cp(o_t[h0:h1, :], s_ps[h0:h1, :])
                        dma(
                            out_h[b, xc * P + h0:xc * P + h1, :], o_t[h0:h1, :]
                        )
                else:
                    nc.scalar.activation(o_t[:], s_ps[:], ACT.Copy)
                    nc.scalar.dma_start(out_h[b, xc * P:(xc + 1) * P, :], o_t[:])

        g2 = gpool.tile([P, KC, 2 * N], bf, name="g0", tag="g")
        s1_phase_a0(g2)
        step1_pair(0, g2, (1,))
        vec_pair(0)
        step2(0, g2, rank2_first=False)
        step2(1, g2)
        for p in range(1, NP):
            vec_pair(p)
            g2 = gpool.tile([P, KC, 2 * N], bf, name=f"g{p}", tag="g")
            step1_pair(p, g2, (0, 1))
            step2(2 * p, g2)
            step2(2 * p + 1, g2)

    nc.finalize()
    return nc


def kernel(D, H, U, W, _trace=False):
    global LAST_RESULT
    _ensure_axon_ntff_hook()
    from concourse.bass_utils import run_bass_kernel_spmd

    D = np.asarray(D, dtype=np.float32)
    H = np.asarray(H, dtype=np.float32)
    U = np.asarray(U, dtype=np.float32)
    W = np.asarray(W, dtype=np.float32)

    # ---- host-side layout / dtype prep (no math beyond the W[d] scalar) ----
    # dtr[b, p, c, x] = D[b, x, c*128+p]  (D^T, chunked along the contraction dim)
    DT = D.transpose(0, 2, 1).astype(BF16)  # [B, DD, N]
    dtr = np.ascontiguousarray(DT.reshape(B, KC, P, N).transpose(0, 2, 1, 3))
    # htr[pair, jc, p, b01*N+n] = H[2*pair+b01, n, jc*128+p]; each (pair, jc)
    # chunk is one contiguous 512 KB block for big-line DMA
    HT = H.transpose(0, 2, 1).astype(BF16)  # [B, DD, N]
    htr = np.ascontiguousarray(
        HT.reshape(B // 2, 2, KC, P, N).transpose(0, 2, 3, 1, 4)
        .reshape(B // 2, KC, P, 2 * N)
    )
    # ujt[p, jc, i] = U[i, jc*128+p]
    U1T = U[:, :DD].T  # [j, i]
    ujt = np.ascontiguousarray(U1T.reshape(KC, P, DD).transpose(1, 0, 2)).astype(BF16)
    # vpr[p, c, :] = (wh, wd)[c*128+p] ; u2r[p, c] = u2[c*128+p] (fp32, folded into G)
    vp = np.stack([W[:DD], W[DD + 1:]], axis=1)  # [DD, 2]
    vpr = np.ascontiguousarray(vp.reshape(KC, P, 2).transpose(1, 0, 2)).astype(BF16)
    u2r = np.ascontiguousarray(U[:, DD].reshape(KC, P).T).astype(np.float32)
    c_const = float(W[DD])
    # (scale_l, bias_l, scale_r, bias_r) per partition row:
    # lvec row0 = rowH*1+0, row1 = junk*0+1 ; rvec row0 = junk*0+1, row1 = colD*1+c
    cst = np.array(
        [[1.0, 0.0, 0.0, 1.0], [0.0, 1.0, 1.0, c_const]], dtype=np.float32
    )

    nc = _build_bass(c_const)

    in_maps = []
    for c in range(NCORES):
        sl = slice(c * BPC, (c + 1) * BPC)
        slp = slice(c * NP, (c + 1) * NP)
        in_maps.append({
            "dtr": dtr[sl],
            "htr": htr[slp],
            "ujt": ujt,
            "vpr": vpr,
            "u2r": u2r,
            "cst": cst,
        })

    try:
        res = run_bass_kernel_spmd(
            nc, in_maps, core_ids=list(range(NCORES)), trace=_trace,
        )
    except Exception:
        # transient device errors (e.g. NRT_EXEC_UNIT_UNRECOVERABLE) usually
        # clear on retry
        res = run_bass_kernel_spmd(
            nc, in_maps, core_ids=list(range(NCORES)), trace=_trace,
        )
    LAST_RESULT = res

    out = np.concatenate([r["out"] for r in res.results], axis=0)
    return np.ascontiguousarray(out.astype(np.float32))


if __name__ == "__main__":
    rng = np.random.default_rng(0)
    D = rng.standard_normal((B, N, DD), dtype=np.float32)
    H = rng.standard_normal((B, N, DD), dtype=np.float32)
    U = (rng.standard_normal((DD, DD + 1)) * 0.02).astype(np.float32)
    W = (rng.standard_normal((2 * DD + 1,)) * 0.02).astype(np.float32)
    out = kernel(D=D, H=H, U=U, W=W)
    print(out.shape, out.dtype)
